# revision 94
# baseline (speedup 1.0000x reference)
"""BiGAT (2-layer GAT, PyG-style with self-loops) on 8 Trainium2 NeuronCores.

Strategy: partition nodes (and their incoming edges) by destination across 8
cores. Edges are sorted by dst on the host and packed into a uniform
blocks-x-chunks structure (one SPMD program serves all cores).

Per layer:
  node stage : xh = x @ W (PE), attention dot-products via block-diagonal
               matmul; compact per-node rows [xh | a_src | a_dst] (144/66
               cols) written to a local DRAM table; a contiguous AllGather
               replicates the compact table, then one local strided DMA
               re-strides it into gatherable 512B/256B rows (collectives
               reject strided APs, local DMA does not).
  edge stage : dma_gather of table rows by src (calls capped at 1024 descs,
               the hard SWDGE ring limit); per-edge a_dst comes from tiny PE
               matmuls against a host-shipped fp8 transposed one-hot (ST)
               instead of a dst-side gather, copied out of PSUM to bf16 so
               the pre-collective pipeline recycles PSUM; e =
               lrelu(a_src+a_dst); ex = exp(e) (softmax max-shift skipped --
               scores are O(10) so exp cannot overflow); msg = xh_src * ex
               into a contiguous [msg|ex] tile; one-hot scatter matmuls
               accumulate [msg|ex] into PSUM per 125-dst block; epilogue
               divides by the summed ex, adds bias. PREB/PREB2 blocks of
               dst-side work are pre-issued to overlap each AllGather; the
               A/B gather tables are separate tensors so srcA gathers start
               after the A-half restride while B restrides concurrently.

DVE 2x mode: xh features are laid out c-major (c*HEADS+h) so the per-head
broadcasts in msg/epilogue have a packed innermost dim; the edge-side
one-hot S is built K-innermost ([p, w, k]) against a materialized iota so
the dl broadcast sits mid-AP; scatter matmuls take lhsT with free-dim
stride K. Host permutes W1 cols / AA1 rows / b1 / W2 rows to match.

Host-side layout: a greedy LPT permutation packs nodes into 125-dst blocks
with balanced in-degree, then whole blocks are swapped between the two
src-id halves (split at ~55.5%) until every block's per-half edge count
fits the global chunk counts Ka/Kb (each <=2048 descs, one gather call).

dma_gather constraints honored: int16 indices (src tables split at HALF),
row strides and elem sizes multiples of 256B, indices wrapped [16, n/16]
and replicated to 128 partitions.
"""
import sys

sys.path.insert(0, "/opt/trn_rl_repo")

import numpy as np

from concourse import bass, mybir
import concourse.bacc as bacc
import concourse.tile as tile
from concourse.masks import make_identity

F32 = mybir.dt.float32
I16 = mybir.dt.int16
BF16 = True                      # table/gather dtype (False -> float32)
TD = mybir.dt.bfloat16 if BF16 else F32
import ml_dtypes
TNP = ml_dtypes.bfloat16 if BF16 else np.float32

# ---------------- problem constants (hardcoded per contract) ----------------
N_NODES = 50000
N_EDGES = 800000
IN_C, HID_C, OUT_C, HEADS = 128, 16, 64, 8
NEG_SLOPE = 0.2
N_CORES = 8

# ---------------- sharding / tiling parameters ----------------
BLK = 125       # dst nodes per edge-stage block (<=128 for one-hot)
W = 128         # one-hot width (full block; PE col base must be 0)
P1W = 256 if BF16 else 192   # [xh(128) | a_src(8) | a_dst(8) | pad]
P2W = 128                    # [xh2(64) | a_src2(1) | a_dst2(1) | pad]
EPS = 1e-16
NQ = 4          # SWDGE queue count (1 = sim-compatible)
TIME_MODE = "full"   # "full" | "noedge" (skip per-block edge loops)
LRELU_ACT = False  # ACT Lrelu gave wrong results on HW; keep DVE version
PAD = 999.0     # dls value for pad edges (outside any window)
RING2048 = False  # raise SWDGE ring to 2048 descs (scratch 32KB) -> 1-call srcA
SCRATCH = 32768 if RING2048 else 16384
PREB = 12       # blocks of dst-side work pre-issued before the AllGather
PREB2 = 12      # deeper pre-issue for the L2 stage (more SBUF headroom)
COMPACT2 = True  # L2: AllGather a compact [NP,66] table, restride locally
COMPACT1 = True  # back-to-back REP=8 HW runs: compact L1 AllGather+restride
                 # is ~90us faster than the full-row AllGather
COLL_COLS = False  # strided AllGather: rejected by BIR verifier on HW
ST8 = True      # ship the transposed one-hot in fp8 (values are 0/1, exact)
TS = mybir.dt.float8e4
TSNP = ml_dtypes.float8_e4m3
XH8 = False     # fp8 xh in L1 table: rel err 2.3e-2 > 2e-2 gate; keep bf16
U8 = mybir.dt.uint8
FP8 = mybir.dt.float8e4

# c-major feature permutation: new position c*HEADS+h <- old h*HID_C+c
PERM_CM = np.array([[h * HID_C + c for h in range(HEADS)]
                    for c in range(HID_C)]).reshape(-1)


def _wrap16(idx):
    """[L] int array -> dma_gather wrapped layout [128, L//16] int16."""
    L = len(idx)
    w = idx.reshape(L // 16, 16).T
    return np.tile(w, (8, 1)).astype(np.int16)


def _balance_perm(src, dst, n_nodes, split_blocks):
    """LPT-pack nodes into BLK-sized dst blocks balancing in-degree (so the
    max per-block edge count hugs the mean), then swap whole blocks between
    the two src-id halves until every block's per-half edge counts fit the
    smallest possible chunk counts (minimizes Ka/Kb)."""
    import heapq
    deg = np.bincount(dst, minlength=n_nodes) + 1   # + self-loop
    nblk = n_nodes // BLK
    fill = np.zeros(nblk, np.int64)
    assign = np.empty(n_nodes, np.int64)
    heap = [(0, b) for b in range(nblk)]
    heapq.heapify(heap)
    for v in np.argsort(-deg, kind="stable"):
        while True:
            load, b = heapq.heappop(heap)
            if fill[b] < BLK:
                break
        assign[v] = b
        fill[b] += 1
        if fill[b] < BLK:
            heapq.heappush(heap, (load + int(deg[v]), b))

    # block-level half balance: which blocks sit in src-id half A decides
    # every block's a/b edge split; swap A<->B blocks to fit CAP per half
    bs, bd = assign[src], assign[dst]
    E = np.zeros((nblk, nblk), np.int32)
    np.add.at(E, (bs, bd), 1)
    # self loops: src block == dst block per node
    np.add.at(E, (assign, assign), 1)
    tot = E.sum(0)
    half = np.zeros(nblk, bool)
    half[:split_blocks] = True
    a = E[half].sum(0)
    # asymmetric caps: both halves must fit one 2048-desc gather call
    CAP_A, CAP_B = 16 * 128, 8 * 128
    rng = np.random.default_rng(0)

    def cost(av):
        return (np.maximum(av - CAP_A, 0).sum()
                + np.maximum((tot - av) - CAP_B, 0).sum())

    c = cost(a)
    it = 0
    idxA = np.flatnonzero(half)
    idxB = np.flatnonzero(~half)
    while c > 0 and it < 40000:
        it += 1
        # candidate swaps (A-block <-> B-block); keep the best of a sample
        cand1 = idxA[rng.integers(0, len(idxA), 24)]
        cand2 = idxB[rng.integers(0, len(idxB), 24)]
        a2 = a[None, :] + (E[cand2] - E[cand1])
        c2 = (np.maximum(a2 - CAP_A, 0).sum(1)
              + np.maximum((tot[None, :] - a2) - CAP_B, 0).sum(1))
        j = int(np.argmin(c2))
        if c2[j] < c or (c2[j] == c and rng.random() < 0.05):
            a, c = a2[j], int(c2[j])
            B1, B2 = int(cand1[j]), int(cand2[j])
            half[B1], half[B2] = False, True
            idxA[idxA == B1] = B2
            idxB[idxB == B2] = B1
    # relabel blocks: half-A blocks first
    blk_order = np.concatenate([np.flatnonzero(half), np.flatnonzero(~half)])
    blk_rank = np.empty(nblk, np.int64)
    blk_rank[blk_order] = np.arange(nblk)
    key = blk_rank[assign]
    order = np.argsort(key, kind="stable")
    perm = np.empty(n_nodes, np.int64)
    perm[order] = np.arange(n_nodes)
    return perm


def _host_prep(x, edge_index, W1, att_src1, att_dst1, b1, W2, att_src2,
               att_dst2, b2, n_nodes=N_NODES, n_cores=N_CORES):
    """Sort/pack edges, build per-core input maps and compile-time params."""
    NP = n_nodes // n_cores
    NB = NP // BLK
    assert NB * BLK == NP
    nblk_tot0 = n_nodes // BLK
    # src-id table split: ~55.5% of nodes in table A so per-block a/b edge
    # counts fit Ka<=16 / Kb<=8 chunks (each half one gather call)
    SPLIT_BLOCKS = min(int(round(nblk_tot0 * 0.555)), 32767 // BLK)
    SPLIT_BLOCKS = max(SPLIT_BLOCKS, nblk_tot0 - 32767 // BLK)
    HALF = SPLIT_BLOCKS * BLK
    assert HALF < 32768 and (n_nodes - HALF) < 32768 and NP < 32768

    perm = _balance_perm(np.asarray(edge_index[0]),
                         np.asarray(edge_index[1]), n_nodes, SPLIT_BLOCKS)
    src = perm[np.concatenate([np.asarray(edge_index[0]), np.arange(n_nodes)])]
    dst = perm[np.concatenate([np.asarray(edge_index[1]), np.arange(n_nodes)])]
    order = np.argsort(dst, kind="stable")
    src = src[order].astype(np.int64)
    dst = dst[order].astype(np.int64)

    nblk_tot = n_cores * NB
    blk_of = dst // BLK
    is_b = src >= HALF
    # two edge groups per block (src-half a/b); chunk counts are global
    # maxima so the SPMD program shape is uniform across cores
    grp = is_b.astype(np.int64)
    cnt = np.zeros((nblk_tot, 2), np.int64)
    np.add.at(cnt, (blk_of, grp), 1)
    Ka = int(np.ceil(cnt[:, 0].max() / 128))
    Kb = int(np.ceil(cnt[:, 1].max() / 128))
    K = Ka + Kb
    gbase = np.array([0, Ka], np.int64)

    # order edges by (block, group, dloc); stable keeps dst-sorted order
    order2 = np.lexsort((grp, blk_of))
    src, dst, grp = src[order2], dst[order2], grp[order2]
    starts = np.concatenate([[0], np.cumsum(cnt.sum(1))]).astype(np.int64)

    # per-block padded arrays in chunk-group order
    srcA = np.zeros((nblk_tot, Ka * 128), np.int64)      # pad -> row 0
    srcB = np.zeros((nblk_tot, Kb * 128), np.int64)
    dls = np.full((nblk_tot, K * 128), PAD, np.float32)  # block-local dst
    for b in range(nblk_tot):
        s, e = starts[b], starts[b + 1]
        sb, db, gb = src[s:e], dst[s:e], grp[s:e]
        for g in range(2):
            m = gb == g
            n = int(m.sum())
            if n == 0:
                continue
            sl = np.arange(n)
            slot = gbase[g] * 128 + sl        # global slot in [0, K*128)
            dls[b, slot] = db[m] - b * BLK
            if g == 0:
                srcA[b, sl] = sb[m]
            else:
                srcB[b, sl] = sb[m] - HALF

    # shared (replicated) weights, permuted to the c-major feature order
    AA1 = np.zeros((128, 16), np.float32)
    asrc1 = np.asarray(att_src1, np.float32)
    adst1 = np.asarray(att_dst1, np.float32)
    for h in range(HEADS):
        AA1[16 * h:16 * (h + 1), h] = asrc1[h]
        AA1[16 * h:16 * (h + 1), 8 + h] = adst1[h]
    AA2 = np.stack([np.asarray(att_src2, np.float32)[0],
                    np.asarray(att_dst2, np.float32)[0]], axis=1)  # [64, 2]
    # IOTWT[p, w*K+k] = w  (K-innermost iota for the one-hot build)
    iotwt = np.repeat(np.arange(W), K)[None, :].repeat(128, 0)
    shared = {
        "W1": np.asarray(W1, np.float32)[:, PERM_CM],
        "AA1": AA1[PERM_CM, :],
        "B1": np.tile(np.asarray(b1, TNP)[PERM_CM], (128, 1)),
        "W2": np.asarray(W2, TNP)[PERM_CM, :],
        "AA2": AA2,
        "B2": np.tile(np.asarray(b2, np.float32), (128, 1)),
        "IOTWT": iotwt.astype(TNP),                      # [128, W*K]
    }

    xp = np.empty_like(np.asarray(x, np.float32))
    xp[perm] = np.asarray(x, np.float32)                 # permuted rows
    xT = np.ascontiguousarray(xp.T)                      # [128, N]

    in_maps = []
    for c in range(n_cores):
        lo = c * NB
        # per-block idx tensor: [a-idxs | b-idxs] wrapped
        idx = np.stack([
            np.concatenate([_wrap16(srcA[lo + b]), _wrap16(srcB[lo + b])],
                           axis=1)
            for b in range(NB)])
        dlw = dls[lo:lo + NB].reshape(NB, K, 128).transpose(0, 2, 1)
        # transposed one-hot ST[b][d, k*128+j] = (dls[b, k*128+j] == d)
        stf = (dls[lo:lo + NB, None, :] ==
               np.arange(128, dtype=np.float32)[None, :, None])
        m = dict(shared)
        m["xT"] = np.ascontiguousarray(xT[:, c * NP:(c + 1) * NP])
        m["IDX"] = np.ascontiguousarray(idx)
        m["DLOC"] = np.ascontiguousarray(dlw.astype(TNP))
        m["STF"] = np.ascontiguousarray(stf.astype(TSNP if ST8 else TNP))
        in_maps.append(m)

    prm = dict(NP=NP, NB=NB, K=K, Ka=Ka, Kb=Kb, perm=perm,
               n_nodes=n_nodes, n_cores=n_cores, HALF=HALF)
    return in_maps, prm


def _build_program(prm, repeat=1):
    NP, NB, K, Ka, Kb = prm["NP"], prm["NB"], prm["K"], prm["Ka"], prm["Kb"]
    HALF = prm["HALF"]
    n_nodes, n_cores = prm["n_nodes"], prm["n_cores"]
    RG = [list(range(n_cores))]
    CW = (Ka + Kb) * 8  # idx tensor cols (wrapped, 8 per chunk)

    assert Kb <= 8 and Ka <= 16
    nc = bacc.Bacc("TRN2", target_bir_lowering=False, debug=False,
                   num_devices=n_cores, num_swdge_queues=NQ,
                   dynamic_dma_scratch_size=SCRATCH)
    qn = [0]  # round-robin SWDGE queue assignment for gathers

    def next_q():
        qn[0] += 1
        return qn[0] % NQ

    # inputs
    xT = nc.dram_tensor("xT", [128, NP], F32, kind="ExternalInput")
    W1 = nc.dram_tensor("W1", [128, 128], F32, kind="ExternalInput")
    AA1 = nc.dram_tensor("AA1", [128, 16], F32, kind="ExternalInput")
    B1 = nc.dram_tensor("B1", [128, 128], TD, kind="ExternalInput")
    W2 = nc.dram_tensor("W2", [128, 64], TD, kind="ExternalInput")
    AA2 = nc.dram_tensor("AA2", [64, 2], F32, kind="ExternalInput")
    B2 = nc.dram_tensor("B2", [128, 64], F32, kind="ExternalInput")
    IOTWT = nc.dram_tensor("IOTWT", [128, W * K], TD, kind="ExternalInput")
    IDX = nc.dram_tensor("IDX", [NB, 128, CW], I16, kind="ExternalInput")
    DLOC = nc.dram_tensor("DLOC", [NB, 128, K], TD, kind="ExternalInput")
    STF = nc.dram_tensor("STF", [NB, 128, K * W], TS if ST8 else TD,
                         kind="ExternalInput")
    OUT = nc.dram_tensor("out", [NP, OUT_C], F32, kind="ExternalOutput")
    # internal DRAM
    P1T = U8 if XH8 else TD
    if COMPACT1:
        # split A/B tables: srcA gathers start after the A-half restride
        # while the B-half restride still runs (on another queue)
        P1L = nc.dram_tensor("P1L", [NP, 144], P1T)
        P1Fc = nc.dram_tensor("P1Fc", [n_nodes, 144], P1T,
                              addr_space="Shared")
        P1FA = nc.dram_tensor("P1FA", [HALF, P1W], P1T)
        P1FB = nc.dram_tensor("P1FB", [n_nodes - HALF, P1W], P1T)
    else:
        P1L = nc.dram_tensor("P1L", [NP, P1W], P1T)
        P1F = nc.dram_tensor("P1F", [n_nodes, P1W], P1T, addr_space="Shared")
    if COMPACT2:
        P2L = nc.dram_tensor("P2L", [NP, 66], TD)
        P2Fc = nc.dram_tensor("P2Fc", [n_nodes, 66], TD, addr_space="Shared")
        P2FA = nc.dram_tensor("P2FA", [HALF, P2W], TD)
        P2FB = nc.dram_tensor("P2FB", [n_nodes - HALF, P2W], TD)
    else:
        P2L = nc.dram_tensor("P2L", [NP, P2W], TD)
        P2F = nc.dram_tensor("P2F", [n_nodes, P2W], TD, addr_space="Shared")

    mm = mybir.AluOpType
    ACT = mybir.ActivationFunctionType

    from contextlib import ExitStack
    with tile.TileContext(nc) as tc, ExitStack() as ctx:
        cst = ctx.enter_context(tc.tile_pool(name="cst", bufs=1))
        W1t = cst.tile([128, 128], F32)
        AA1t = cst.tile([128, 16], F32)
        B1t = cst.tile([128, 128], TD)
        W2t = cst.tile([128, 64], TD)
        AA2t = cst.tile([64, 2], F32)
        B2t = cst.tile([128, 64], F32)
        IOTWTt = cst.tile([128, W * K], TD)
        IDENT = cst.tile([128, 128], F32)
        IDENTB = cst.tile([128, 128], TD)
        for t, d in ((W1t, W1), (AA1t, AA1), (B1t, B1), (W2t, W2),
                     (AA2t, AA2), (B2t, B2), (IOTWTt, IOTWT)):
            nc.sync.dma_start(out=t[:], in_=d[:, :])
        make_identity(nc, IDENT[:])
        make_identity(nc, IDENTB[:])
        iotwt3 = IOTWTt[:].rearrange("p (w k) -> p w k", k=K)
        # index / dloc tables persist across both edge stages
        ixall = cst.tile([128, NB * CW], I16)
        nc.sync.dma_start(
            out=ixall[:].rearrange("p (b w) -> p b w", w=CW),
            in_=IDX[:, :, :].rearrange("b p w -> p b w"))
        dlall = cst.tile([128, NB * K], TD)
        nc.sync.dma_start(
            out=dlall[:].rearrange("p (b w) -> p b w", w=K),
            in_=DLOC[:, :, :].rearrange("b p w -> p b w"))

        # body may be repeated for differential benchmarking
        for _rep in range(repeat):
            hT, free_hT = tc.tile([128, NP], TD, name="hT")  # h^T, persists L1

            # ---------------- stage A: L1 node stage ----------------
            with tc.tile_pool(name="pa", bufs=4) as pa, \
                 tc.tile_pool(name="ppa", bufs=4, space="PSUM") as ppa:
                for it, c0 in enumerate(range(0, NP, 128)):
                    nn = min(128, NP - c0)
                    xt = pa.tile([128, 128], F32, tag="xt")
                    nc.sync.dma_start(out=xt[:, :nn], in_=xT[:, c0:c0 + nn])
                    pm = ppa.tile([128, 128], F32, tag="pp")
                    nc.tensor.matmul(pm[:, :nn], lhsT=W1t[:], rhs=xt[:, :nn],
                                     start=True, stop=True)
                    xhT = pa.tile([128, 128], F32, tag="xhT")
                    nc.scalar.copy(out=xhT[:, :nn], in_=pm[:, :nn])
                    pm2 = ppa.tile([16, 128], F32, tag="pp")
                    nc.tensor.matmul(pm2[:, :nn], lhsT=AA1t[:], rhs=xhT[:, :nn],
                                     start=True, stop=True)
                    aaT = pa.tile([16, 128], F32, tag="aaT")
                    nc.scalar.copy(out=aaT[:, :nn], in_=pm2[:, :nn])
                    pt = ppa.tile([128, 128], F32, tag="pp")
                    nc.tensor.transpose(pt[:nn, :], xhT[:, :nn], IDENT[:])
                    row = pa.tile([128, P1W], P1T, tag="row")
                    if XH8:
                        nc.scalar.copy(out=row[:nn, 0:128].bitcast(FP8),
                                       in_=pt[:nn, :])
                    else:
                        nc.scalar.copy(out=row[:nn, 0:128], in_=pt[:nn, :])
                    pt2 = ppa.tile([128, 16], F32, tag="pp")
                    nc.tensor.transpose(pt2[:nn, :], aaT[:, :nn], IDENT[:16, :16])
                    if XH8:
                        nc.vector.memset(row[:, 160:], 0)
                        nc.scalar.copy(out=row[:nn, 128:160].bitcast(TD),
                                       in_=pt2[:nn, :])
                    else:
                        if not COMPACT1:
                            nc.vector.memset(row[:, 144:], 0.0)
                        nc.scalar.copy(out=row[:nn, 128:144], in_=pt2[:nn, :])
                    if COMPACT1:
                        nc.sync.dma_start(out=P1L[c0:c0 + nn, :],
                                          in_=row[:nn, 0:144])
                    else:
                        nc.sync.dma_start(out=P1L[c0:c0 + nn, :],
                                          in_=row[:nn, :])

            # ---------------- L1 edge stage (+ fused L2 node stage) ----------------
            with tc.tile_pool(name="gma", bufs=5) as gmap, \
                 tc.tile_pool(name="gmb", bufs=5) as gmbp, \
                 tc.tile_pool(name="stp", bufs=PREB) as stp, \
                 tc.tile_pool(name="adp", bufs=4) as adp, \
                 tc.tile_pool(name="adw", bufs=PREB) as adwp, \
                 tc.tile_pool(name="sml", bufs=6) as sml, \
                 tc.tile_pool(name="mx", bufs=4) as mxp, \
                 tc.tile_pool(name="sp", bufs=PREB) as spp, \
                 tc.tile_pool(name="hb", bufs=4) as hbp, \
                 tc.tile_pool(name="a2", bufs=4) as a2p, \
                 tc.tile_pool(name="ps1", bufs=3, space="PSUM") as ps1p, \
                 tc.tile_pool(name="psad", bufs=2, space="PSUM") as psadp, \
                 tc.tile_pool(name="psa2", bufs=3, space="PSUM") as psa2p:
                if RING2048:
                    ra = [(0, Ka, Ka * 128)]
                else:
                    ra = [(0, 8, 1024), (8, Ka, (Ka - 8) * 128)] \
                        if Ka > 8 else [(0, Ka, Ka * 128)]
                rb = [(0, Kb, Kb * 128)]

                def gcalls(ix, out3, in_ap, ixo, regs, elem, estep=None):
                    kw = {} if estep is None else {"elem_step": estep}
                    for c0, c1, rv in regs:
                        nc.gpsimd.dma_gather(
                            out_ap=out3[:, c0:c1, :], in_ap=in_ap,
                            idxs_ap=ix[:, (ixo + c0) * 8:(ixo + c1) * 8],
                            num_idxs=(c1 - c0) * 128,
                            num_idxs_reg=rv, elem_size=elem,
                            queue_num=next_q(), **kw)

                # dst-side work (no P1F dep): ST load, block a_dst rows,
                # per-edge a_dst matmuls, one-hot build
                def dst_work(b, psw):
                    stt = stp.tile([128, K * W], TS if ST8 else TD, tag="stt")
                    eng = nc.sync if b % 2 else nc.scalar
                    eng.dma_start(out=stt[:], in_=STF[b, :, :])
                    adb = adp.tile([128, psw], TD, tag="adb")
                    n0 = b * BLK
                    if XH8:
                        nc.sync.dma_start(
                            out=adb[:BLK, :],
                            in_=P1L[n0:n0 + BLK, 144:160].bitcast(TD))
                    else:
                        nc.sync.dma_start(
                            out=adb[:BLK, :],
                            in_=P1L[n0:n0 + BLK, 136:144])
                    psad = psadp.tile([128, K * psw], F32, tag="psad")
                    for k in range(K):
                        nc.tensor.matmul(
                            psad[:, k * psw:(k + 1) * psw],
                            lhsT=stt[:BLK, k * W:(k + 1) * W],
                            rhs=adb[:BLK, :], start=True, stop=True)
                    adw = adwp.tile([128, K * psw], TD, tag="adw")
                    nc.scalar.copy(out=adw[:], in_=psad[:])
                    S = spp.tile([128, W * K], TD, tag="S")
                    dl = dlall[:, b * K:(b + 1) * K]
                    nc.vector.tensor_tensor(
                        out=S[:].rearrange("p (w k) -> p w k", k=K),
                        in0=iotwt3,
                        in1=dl.rearrange("p (o k) -> p o k",
                                         o=1).to_broadcast([128, W, K]),
                        op=mm.is_equal)
                    return stt, adw, S

                NEDGE = NB if TIME_MODE != "noedge" else 0
                pre = {b: dst_work(b, 8) for b in range(min(PREB, NEDGE))}

                if COMPACT1:
                    nc.gpsimd.collective_compute(
                        "AllGather", mm.bypass, replica_groups=RG,
                        ins=[P1L[:, :]], outs=[P1Fc[:, :]])
                    # local re-stride into gatherable 512B rows; A first so
                    # srcA gathers start while the B half still restrides
                    nc.sync.dma_start(out=P1FA[:, 0:144],
                                      in_=P1Fc[0:HALF, :])
                    nc.scalar.dma_start(out=P1FB[:, 0:144],
                                        in_=P1Fc[HALF:n_nodes, :])
                    apA1, apB1 = P1FA[:, :], P1FB[:, :]
                else:
                    nc.gpsimd.collective_compute(
                        "AllGather", mm.bypass, replica_groups=RG,
                        ins=[P1L[:, :]], outs=[P1F[:, :]])
                    apA1, apB1 = P1F[0:HALF, :], P1F[HALF:n_nodes, :]

                for b in range(NEDGE):
                        ix = ixall[:, b * CW:(b + 1) * CW]
                        gmA = gmap.tile([128, Ka * P1W], P1T, tag="gmA")
                        gmA3 = gmA[:].rearrange("p (r w) -> p r w", w=P1W)
                        gcalls(ix, gmA3, apA1, 0, ra, P1W)
                        gmB = gmbp.tile([128, Kb * P1W], P1T, tag="gmB")
                        gmB3 = gmB[:].rearrange("p (r w) -> p r w", w=P1W)
                        gcalls(ix, gmB3, apB1, Ka, rb, P1W)
                        _, psad, S = (pre.pop(b) if b in pre
                                      else dst_work(b, 8))
                        S3 = S[:].rearrange("p (w k) -> p w k", k=K)
                        if XH8:
                            asA = gmA3[:, :, 128:144].bitcast(TD)
                            asB = gmB3[:, :, 128:144].bitcast(TD)
                            xhA = gmA3[:, :, 0:128].bitcast(FP8)
                            xhB = gmB3[:, :, 0:128].bitcast(FP8)
                        else:
                            asA = gmA3[:, :, 128:136]
                            asB = gmB3[:, :, 128:136]
                            xhA = gmA3[:, :, 0:128]
                            xhB = gmB3[:, :, 0:128]
                        ea = sml.tile([128, K * 8], TD, tag="ea")
                        ea3 = ea[:].rearrange("p (r w) -> p r w", w=8)
                        # e = a_src[src] + a_dst[dst]; a-chunks then b-chunks
                        nc.vector.tensor_tensor(
                            out=ea3[:, 0:Ka, :],
                            in0=asA,
                            in1=psad[:].rearrange(
                                "p (r w) -> p r w", w=8)[:, 0:Ka, :],
                            op=mm.add)
                        nc.vector.tensor_tensor(
                            out=ea3[:, Ka:K, :],
                            in0=asB,
                            in1=psad[:].rearrange(
                                "p (r w) -> p r w", w=8)[:, Ka:K, :],
                            op=mm.add)
                        if LRELU_ACT:
                            nc.scalar.activation(out=ea[:], in_=ea[:],
                                                 func=ACT.Lrelu,
                                                 alpha=NEG_SLOPE)
                        else:
                            tl = sml.tile([128, K * 8], TD, tag="tl")
                            nc.vector.tensor_scalar_mul(tl[:], ea[:], NEG_SLOPE)
                            nc.vector.tensor_tensor(out=ea[:], in0=ea[:],
                                                    in1=tl[:], op=mm.max)
                        # [msg | ex] tile: gathers stay read-only; matmul rhs
                        # gets a tight contiguous 136-col layout
                        mx = mxp.tile([128, K * 136], TD, tag="mx")
                        mx3 = mx[:].rearrange("p (r w) -> p r w", w=136)
                        nc.scalar.activation(
                            out=mx3[:, :, 128:136],
                            in_=ea3[:, :, :], func=ACT.Exp)
                        # msg = xh * ex; c-major keeps the head broadcast
                        # innermost-packed (DVE 2x)
                        nc.vector.tensor_tensor(
                            out=mx3[:, 0:Ka, 0:128].rearrange(
                                "p k (c h) -> p k c h", h=8),
                            in0=xhA.rearrange("p k (c h) -> p k c h", h=8),
                            in1=mx3[:, 0:Ka, 128:136].rearrange(
                                "p k (o h) -> p k o h", o=1).to_broadcast(
                                [128, Ka, 16, 8]),
                            op=mm.mult)
                        nc.vector.tensor_tensor(
                            out=mx3[:, Ka:K, 0:128].rearrange(
                                "p k (c h) -> p k c h", h=8),
                            in0=xhB.rearrange("p k (c h) -> p k c h", h=8),
                            in1=mx3[:, Ka:K, 128:136].rearrange(
                                "p k (o h) -> p k o h", o=1).to_broadcast(
                                [128, Kb, 16, 8]),
                            op=mm.mult)
                        ps = ps1p.tile([128, 136], F32, tag="ps")
                        for k in range(K):
                            nc.tensor.matmul(
                                ps[:],
                                lhsT=S3[:, :, k],
                                rhs=mx[:, k * 136:(k + 1) * 136],
                                start=(k == 0), stop=(k == K - 1))
                        # epilogue: h = psum[:, :128] / den + b1 ; elu
                        rd = sml.tile([128, 8], F32, tag="rd")
                        nc.vector.tensor_scalar_add(rd[:], ps[:, 128:136], EPS)
                        nc.vector.reciprocal(rd[:], rd[:])
                        hb = hbp.tile([128, 128], TD, tag="hb")
                        nc.vector.tensor_tensor(
                            out=hb[:].rearrange("p (c h) -> p c h", h=8),
                            in0=ps[:, 0:128].rearrange("p (c h) -> p c h", h=8),
                            in1=rd[:].rearrange("p (o h) -> p o h",
                                                o=1).to_broadcast([128, 16, 8]),
                            op=mm.mult)
                        nc.vector.tensor_tensor(out=hb[:], in0=hb[:], in1=B1t[:],
                                                op=mm.add)
                        tm = hbp.tile([128, 128], TD, tag="tm")
                        nc.vector.tensor_scalar_min(tm[:], hb[:], 0.0)
                        nc.scalar.activation(out=tm[:], in_=tm[:], func=ACT.Exp)
                        nc.vector.tensor_scalar_sub(tm[:], tm[:], 1.0)
                        nc.vector.tensor_tensor(out=hb[:], in0=hb[:], in1=tm[:],
                                                op=mm.max)
                        # fused L2 node stage for this block
                        n0 = b * BLK
                        pt = psa2p.tile([128, 128], TD, tag="pa2")
                        nc.tensor.transpose(pt[:, :BLK], hb[:BLK, :],
                                            IDENTB[:BLK, :BLK])
                        nc.scalar.copy(out=hT[:, n0:n0 + BLK],
                                       in_=pt[:, :BLK])
                        p2m = psa2p.tile([64, 128], F32, tag="pa2")
                        nc.tensor.matmul(p2m[:, :BLK], lhsT=W2t[:],
                                         rhs=hT[:, n0:n0 + BLK],
                                         start=True, stop=True)
                        x2T = a2p.tile([64, 128], F32, tag="x2T")
                        nc.scalar.copy(out=x2T[:, :BLK], in_=p2m[:, :BLK])
                        p2a = psa2p.tile([2, 128], F32, tag="pa2")
                        nc.tensor.matmul(p2a[:, :BLK], lhsT=AA2t[:],
                                         rhs=x2T[:, :BLK], start=True, stop=True)
                        a2T = a2p.tile([2, 128], F32, tag="a2T")
                        nc.scalar.copy(out=a2T[:, :BLK], in_=p2a[:, :BLK])
                        p2t = psa2p.tile([128, 64], F32, tag="pa2")
                        nc.tensor.transpose(p2t[:BLK, :], x2T[:, :BLK],
                                            IDENT[:64, :64])
                        row2 = a2p.tile([128, 66 if COMPACT2 else 128], TD,
                                        tag="row2")
                        nc.scalar.copy(out=row2[:BLK, 0:64], in_=p2t[:BLK, :])
                        p2u = psa2p.tile([128, 2], F32, tag="pa2")
                        nc.tensor.transpose(p2u[:BLK, :], a2T[:, :BLK],
                                            IDENT[:2, :2])
                        if not COMPACT2:
                            nc.vector.memset(row2[:, 66:], 0.0)
                        nc.scalar.copy(out=row2[:BLK, 64:66], in_=p2u[:BLK, :])
                        nc.sync.dma_start(out=P2L[n0:n0 + BLK, :],
                                          in_=row2[:BLK, :])

            free_hT()

            # ---------------- L2 edge stage ----------------
            with tc.tile_pool(name="gma2", bufs=5) as gmap2, \
                 tc.tile_pool(name="gmb2", bufs=5) as gmbp2, \
                 tc.tile_pool(name="stp2", bufs=PREB2) as stp2, \
                 tc.tile_pool(name="adp2", bufs=4) as adp2, \
                 tc.tile_pool(name="adw2", bufs=PREB2) as adwp2, \
                 tc.tile_pool(name="sml2", bufs=4) as sml2, \
                 tc.tile_pool(name="mx2", bufs=4) as mxp2, \
                 tc.tile_pool(name="sp2", bufs=PREB2) as spp2, \
                 tc.tile_pool(name="ob", bufs=4) as obp, \
                 tc.tile_pool(name="ps2", bufs=5, space="PSUM") as ps2p, \
                 tc.tile_pool(name="psad2", bufs=2, space="PSUM") as psadp2:

                if RING2048:
                    ra = [(0, Ka, Ka * 128)]
                else:
                    ra = [(0, 8, 1024), (8, Ka, (Ka - 8) * 128)] \
                        if Ka > 8 else [(0, Ka, Ka * 128)]
                rb = [(0, Kb, Kb * 128)]

                def gcalls(ix, out3, in_ap, ixo, regs, elem, estep=None):
                    kw = {} if estep is None else {"elem_step": estep}
                    for c0, c1, rv in regs:
                        nc.gpsimd.dma_gather(
                            out_ap=out3[:, c0:c1, :], in_ap=in_ap,
                            idxs_ap=ix[:, (ixo + c0) * 8:(ixo + c1) * 8],
                            num_idxs=(c1 - c0) * 128,
                            num_idxs_reg=rv, elem_size=elem,
                            queue_num=next_q(), **kw)

                def dst_work2(b):
                    stt = stp2.tile([128, K * W], TS if ST8 else TD, tag="stt2")
                    eng = nc.sync if b % 2 else nc.scalar
                    eng.dma_start(out=stt[:], in_=STF[b, :, :])
                    adb = adp2.tile([128, 1], TD, tag="adb2")
                    n0 = b * BLK
                    nc.sync.dma_start(out=adb[:BLK, :],
                                      in_=P2L[n0:n0 + BLK, 65:66])
                    psad = psadp2.tile([128, K], F32, tag="psad2")
                    for k in range(K):
                        nc.tensor.matmul(
                            psad[:, k:k + 1],
                            lhsT=stt[:BLK, k * W:(k + 1) * W],
                            rhs=adb[:BLK, :], start=True, stop=True)
                    adw = adwp2.tile([128, K], TD, tag="adw2")
                    nc.scalar.copy(out=adw[:], in_=psad[:])
                    S = spp2.tile([128, W * K], TD, tag="S2")
                    dl = dlall[:, b * K:(b + 1) * K]
                    nc.vector.tensor_tensor(
                        out=S[:].rearrange("p (w k) -> p w k", k=K),
                        in0=iotwt3,
                        in1=dl.rearrange("p (o k) -> p o k",
                                         o=1).to_broadcast([128, W, K]),
                        op=mm.is_equal)
                    return stt, adw, S

                NEDGE = NB if TIME_MODE != "noedge" else 0
                pre = {b: dst_work2(b) for b in range(min(PREB2, NEDGE))}

                if COMPACT2:
                    nc.gpsimd.collective_compute(
                        "AllGather", mm.bypass, replica_groups=RG,
                        ins=[P2L[:, :]], outs=[P2Fc[:, :]])
                    # local re-stride into gatherable 256B rows; A first
                    nc.sync.dma_start(out=P2FA[:, 0:66],
                                      in_=P2Fc[0:HALF, :])
                    nc.scalar.dma_start(out=P2FB[:, 0:66],
                                        in_=P2Fc[HALF:n_nodes, :])
                    apA2, apB2 = P2FA[:, :], P2FB[:, :]
                else:
                    nc.gpsimd.collective_compute(
                        "AllGather", mm.bypass, replica_groups=RG,
                        ins=[P2L[:, :]], outs=[P2F[:, :]])
                    apA2, apB2 = P2F[0:HALF, :], P2F[HALF:n_nodes, :]

                for b in range(NEDGE):
                        ix = ixall[:, b * CW:(b + 1) * CW]
                        gmA = gmap2.tile([128, Ka * P2W], TD, tag="gmA2")
                        gmA3 = gmA[:].rearrange("p (r w) -> p r w", w=P2W)
                        gcalls(ix, gmA3, apA2, 0, ra, P2W)
                        gmB = gmbp2.tile([128, Kb * P2W], TD, tag="gmB2")
                        gmB3 = gmB[:].rearrange("p (r w) -> p r w", w=P2W)
                        gcalls(ix, gmB3, apB2, Ka, rb, P2W)
                        _, psad, S = (pre.pop(b) if b in pre else dst_work2(b))
                        S3 = S[:].rearrange("p (w k) -> p w k", k=K)
                        asA = gmA3[:, :, 64:65].rearrange("p k w -> p (k w)")
                        asB = gmB3[:, :, 64:65].rearrange("p k w -> p (k w)")
                        ea = sml2.tile([128, K], TD, tag="ea2")
                        nc.vector.tensor_tensor(
                            out=ea[:, 0:Ka], in0=asA,
                            in1=psad[:, 0:Ka], op=mm.add)
                        nc.vector.tensor_tensor(
                            out=ea[:, Ka:K], in0=asB,
                            in1=psad[:, Ka:K], op=mm.add)
                        if LRELU_ACT:
                            nc.scalar.activation(out=ea[:], in_=ea[:],
                                                 func=ACT.Lrelu,
                                                 alpha=NEG_SLOPE)
                        else:
                            tl = sml2.tile([128, K], TD, tag="tl2")
                            nc.vector.tensor_scalar_mul(tl[:], ea[:], NEG_SLOPE)
                            nc.vector.tensor_tensor(out=ea[:], in0=ea[:],
                                                    in1=tl[:], op=mm.max)
                        mx = mxp2.tile([128, K * 65], TD, tag="mx2")
                        mx3 = mx[:].rearrange("p (r w) -> p r w", w=65)
                        nc.scalar.activation(
                            out=mx3[:, :, 64:65],
                            in_=ea[:].rearrange("p (k o) -> p k o", o=1),
                            func=ACT.Exp)
                        # msg2 = xh2 * ex, one batched op per half
                        nc.vector.tensor_tensor(
                            out=mx3[:, 0:Ka, 0:64], in0=gmA3[:, :, 0:64],
                            in1=mx3[:, 0:Ka, 64:65].to_broadcast(
                                [128, Ka, 64]),
                            op=mm.mult)
                        nc.vector.tensor_tensor(
                            out=mx3[:, Ka:K, 0:64], in0=gmB3[:, :, 0:64],
                            in1=mx3[:, Ka:K, 64:65].to_broadcast(
                                [128, Kb, 64]),
                            op=mm.mult)
                        ps = ps2p.tile([128, 65], F32, tag="psb")
                        for k in range(K):
                            nc.tensor.matmul(
                                ps[:],
                                lhsT=S3[:, :, k],
                                rhs=mx[:, k * 65:(k + 1) * 65],
                                start=(k == 0), stop=(k == K - 1))
                        rd = sml2.tile([128, 1], F32, tag="rd2")
                        nc.vector.tensor_scalar_add(rd[:], ps[:, 64:65], EPS)
                        nc.vector.reciprocal(rd[:], rd[:])
                        ob = obp.tile([128, 64], F32, tag="ob")
                        nc.vector.tensor_tensor(out=ob[:], in0=ps[:, 0:64],
                                                in1=rd[:].to_broadcast([128, 64]),
                                                op=mm.mult)
                        nc.vector.tensor_tensor(out=ob[:], in0=ob[:], in1=B2t[:],
                                                op=mm.add)
                        n0 = b * BLK
                        nc.sync.dma_start(out=OUT[n0:n0 + BLK, :], in_=ob[:BLK, :])

    nc.compile()
    return nc


def _run(inputs, sim=False):
    in_maps, prm = _host_prep(**inputs)
    nc = _build_program(prm)
    n_cores = prm["n_cores"]
    if sim:
        from concourse.bass_interp import MultiCoreSim
        ms = MultiCoreSim(nc, num_cores=n_cores)
        for c in range(n_cores):
            for k, v in in_maps[c].items():
                ms.cores[c].tensor(k)[:] = v
        ms.simulate()
        outs = [np.array(ms.cores[c].tensor("out")) for c in range(n_cores)]
        return np.concatenate(outs, axis=0)[prm["perm"]], None
    from concourse.bass_utils import run_bass_kernel_spmd
    res = run_bass_kernel_spmd(nc, in_maps, core_ids=list(range(n_cores)))
    outs = [res.results[c]["out"] for c in range(n_cores)]
    return np.concatenate(outs, axis=0)[prm["perm"]], res


def kernel(**inputs):
    out, _ = _run({k: np.asarray(v) for k, v in inputs.items()})
    return out
